# revision 27
# baseline (speedup 1.0000x reference)
"""Trainium2 Bass kernel for BLIP-v2 style retrieval KNN.

Reference computation (per batch b):
    tn  = l2norm(text[b])            # [T, D]
    imn = l2norm(image[b])           # [Z, I, D]
    cos = einsum('td,zid->tzi', tn, imn)
    idx = argmax(cos, axis=-1)       # [T, Z]
    out[b, t, z, :] = image[b, z, idx[t, z], :]   (raw, unnormalized)

Sharding: data-parallel over B across the 8 NeuronCores (one batch per
core, no cross-core communication).

Per-core pipeline (loop over the Z=16 image groups):
  - DMA the [I, D] image group into SBUF, 8 tokens packed per partition
    (token i = p*8 + c) so each partition reads one contiguous 8KB run.
  - per-token 1/||row||: scalar-engine square+accumulate and sqrt, DVE
    reciprocal.  (text normalization is skipped: it scales each score
    row by a positive constant and cannot change the argmax.)
  - scale image rows by 1/||row|| on DVE (per-partition scalar).
  - transpose the scaled image to [D, I'] layout on the TensorEngine
    (I' is a permuted token order; the iota table below encodes the
    permutation so the argmax still yields true token ids).
  - scores[t, i'] via float32r matmuls (textT stationary), with a
    second accumulation pass against the text rounding residual: f32r
    keeps only 11 mantissa bits, and the residual term keeps the
    argmax faithful to the fp32 reference on near-ties.
  - scalar-engine eviction of the scores, row max via DVE
    tensor_scalar max-accumulate, argmax via a fused is_equal*iota
    sum-accumulate on DVE.
  - indirect-DMA gather of the raw image rows (one offset per
    partition, per text chunk), staged per 8-z group, then batched
    stores with 8KB-contiguous runs.

Measured on 8 axon TRN2 cores: rel err 1.5e-2 (8/65536 rows differ from
the fp32 reference argmax on ulp-level near-ties), exec ~286 us.
"""

import numpy as np
from contextlib import ExitStack

import concourse.bass as bass
import concourse.tile as tile
from concourse import bacc
from concourse import mybir
from concourse.bass import IndirectOffsetOnAxis
from concourse.bass_utils import run_bass_kernel_spmd

F32 = mybir.dt.float32
F32R = mybir.dt.float32r
U32 = mybir.dt.uint32

B = 8
T, Z, I, D = 512, 16, 1024, 256
P = 128
TC = T // P  # 4 text chunks
IC = I // P  # 8 tokens per partition / chunks per group
DC = D // P  # 2 contraction chunks
ZG = 8       # z-group size for batched stores

N_CORES = 8


def build_nc():
    nc = bacc.Bacc("TRN2", target_bir_lowering=False, debug=False)
    text_in = nc.declare_dram_parameter("text", [T, D], F32, isOutput=False)
    img_in = nc.declare_dram_parameter("image", [Z, I, D], F32, isOutput=False)
    out_p = nc.declare_dram_parameter("out", [T, Z, D], F32, isOutput=True)

    with ExitStack() as ctx:
        tc = ctx.enter_context(tile.TileContext(nc))

        const_pool = ctx.enter_context(tc.tile_pool(name="const", bufs=1))
        img_pool = ctx.enter_context(tc.tile_pool(name="img", bufs=2))
        imgs_pool = ctx.enter_context(tc.tile_pool(name="imgs", bufs=2))
        imgT_pool = ctx.enter_context(tc.tile_pool(name="imgT", bufs=2))
        sc_pool = ctx.enter_context(tc.tile_pool(name="scsb", bufs=3))
        small_pool = ctx.enter_context(tc.tile_pool(name="small", bufs=2))
        junk_pool = ctx.enter_context(tc.tile_pool(name="junk", bufs=2))
        gath_pool = ctx.enter_context(tc.tile_pool(name="gath", bufs=2))
        idx_pool = ctx.enter_context(tc.tile_pool(name="idx", bufs=2))
        psum_tr = ctx.enter_context(tc.tile_pool(name="ptr", bufs=2, space="PSUM"))
        psum_sc = ctx.enter_context(tc.tile_pool(name="psc", bufs=2, space="PSUM"))

        # ---- constants ----
        # identity matrix for TensorEngine transposes
        iota_free = junk_pool.tile([P, P], F32, tag="if32")
        nc.gpsimd.iota(
            iota_free[:, :], pattern=[[1, P]], channel_multiplier=0,
            allow_small_or_imprecise_dtypes=True,
        )
        iota_part = const_pool.tile([P, 1], F32)
        nc.gpsimd.iota(
            iota_part[:, :], pattern=[[0, 1]], channel_multiplier=1,
            allow_small_or_imprecise_dtypes=True,
        )
        ident = const_pool.tile([P, P], F32)
        nc.gpsimd.tensor_scalar(
            out=ident[:, :],
            in0=iota_free[:, :],
            scalar1=iota_part[:, 0:1],
            scalar2=None,
            op0=mybir.AluOpType.is_equal,
        )
        # token-id table: score column i' = c*128 + j holds token j*IC + c
        iota_f = const_pool.tile([P, IC, P], F32)
        nc.gpsimd.iota(
            iota_f[:, :, :], pattern=[[1, IC], [IC, P]], channel_multiplier=0,
            allow_small_or_imprecise_dtypes=True,
        )

        # ---- text load + transpose (prologue) ----
        # partition p slot c -> t = p*TC + c  (contiguous 4KB per partition)
        text_nat = const_pool.tile([P, TC, D], F32)
        nc.sync.dma_start(
            out=text_nat[:, :, :],
            in_=text_in[:, :].rearrange("(p c) d -> p (c d)", c=TC),
        )
        # textT is rounded to float32r (11-bit mantissa); textT_lo holds the
        # rounding residual so text contributes ~22 exact bits to the scores
        textT = const_pool.tile([P, DC, T], F32R)
        textT_lo = const_pool.tile([P, DC, T], F32R)
        for dc in range(DC):
            ptile = psum_tr.tile([P, T], F32, tag="ptile")
            for c in range(TC):
                # block column j of the transpose is partition j, i.e.
                # true t = j*TC + c lands at textT position c*128 + j
                nc.tensor.transpose(
                    out=ptile[:, c * P : (c + 1) * P],
                    in_=text_nat[:, c, dc * P : (dc + 1) * P],
                    identity=ident[:, :],
                )
            nc.scalar.copy(out=textT[:, dc, :], in_=ptile[:, :])
            nc.vector.tensor_tensor(
                out=textT_lo[:, dc, :],
                in0=ptile[:, :],
                in1=textT[:, dc, :].bitcast(F32),
                op=mybir.AluOpType.subtract,
            )

        img_flat = img_in[:, :, :].rearrange("z i d -> (z i) d")

        # ---- main loop over image groups ----
        for zg in range(Z // ZG):
            gath = gath_pool.tile([P, TC, ZG, D], F32)
            for zz in range(ZG):
                z = zg * ZG + zz
                # token i = p*IC + c at partition p slot c (8KB runs)
                img_nat = img_pool.tile([P, IC, D], F32)
                nc.sync.dma_start(
                    out=img_nat[:, :, :],
                    in_=img_in[z, :, :].rearrange("(p c) d -> p (c d)", c=IC),
                )

                # per-token sum of squares on the scalar engine
                ssq = small_pool.tile([P, IC], F32, tag="ssq")
                sq_junk = junk_pool.tile([P, D], F32, tag="sqj")
                for c in range(IC):
                    nc.scalar.activation(
                        out=sq_junk[:, :],
                        in_=img_nat[:, c, :],
                        func=mybir.ActivationFunctionType.Square,
                        accum_out=ssq[:, c : c + 1],
                    )
                norm = small_pool.tile([P, IC], F32, tag="norm")
                nc.scalar.activation(
                    out=norm[:, :], in_=ssq[:, :],
                    func=mybir.ActivationFunctionType.Sqrt,
                )
                inv_norm = small_pool.tile([P, IC], F32, tag="inv")
                nc.vector.reciprocal(out=inv_norm[:, :], in_=norm[:, :])

                # normalize image rows (per-partition scalar per slot),
                # split across DVE and the scalar engine to balance load
                img_s = imgs_pool.tile([P, IC, D], F32)
                for c in range(IC):
                    if c % 2 == 0:
                        nc.vector.tensor_scalar(
                            out=img_s[:, c, :],
                            in0=img_nat[:, c, :],
                            scalar1=inv_norm[:, c : c + 1],
                            scalar2=None,
                            op0=mybir.AluOpType.mult,
                        )
                    else:
                        nc.scalar.mul(
                            out=img_s[:, c, :],
                            in_=img_nat[:, c, :],
                            mul=inv_norm[:, c : c + 1],
                        )

                # transpose normalized image to [d, i'] layout
                # (column c*128+j holds token j*IC+c); imgT_lo carries the
                # float32r rounding residual for the exact-score correction
                imgT = imgT_pool.tile([P, DC, I], F32R)
                for dc in range(DC):
                    ptile = psum_tr.tile([P, I], F32, tag="ptile")
                    for c in range(IC):
                        nc.tensor.transpose(
                            out=ptile[:, c * P : (c + 1) * P],
                            in_=img_s[:, c, dc * P : (dc + 1) * P],
                            identity=ident[:, :],
                        )
                    nc.scalar.copy(out=imgT[:, dc, :], in_=ptile[:, :])

                # scores + row max + argmax per text chunk
                idxf = idx_pool.tile([P, TC], F32, tag="idxf")
                for tcx in range(TC):
                    stile = psum_sc.tile([P, I], F32)
                    for ih in range(2):
                        # exact scores: t_hi*i_hi + t_lo*i_hi + t_hi*i_lo
                        # (t_lo*i_lo ~ 2^-44 relative, negligible)
                        mms = [
                            (textT, imgT, dc) for dc in range(DC)
                        ] + [
                            (textT_lo, imgT, dc) for dc in range(DC)
                        ]
                        for k, (lh, rh, dc) in enumerate(mms):
                            nc.tensor.matmul(
                                out=stile[:, ih * 512 : (ih + 1) * 512],
                                lhsT=lh[:, dc, tcx * P : (tcx + 1) * P],
                                rhs=rh[:, dc, ih * 512 : (ih + 1) * 512],
                                start=(k == 0),
                                stop=(k == len(mms) - 1),
                            )
                    # evict on scalar engine; row-max via DVE tensor_scalar
                    # accumulate (op1 drives the reduce)
                    scores_sb = sc_pool.tile([P, I], F32)
                    m = small_pool.tile([P, 1], F32, tag="m")
                    rmax_junk = junk_pool.tile([P, I], F32, tag="rmx")
                    nc.scalar.copy(out=scores_sb[:, :], in_=stile[:, :])
                    nc.vector.tensor_scalar(
                        out=rmax_junk[:, :],
                        in0=scores_sb[:, :],
                        scalar1=1.0,
                        scalar2=None,
                        op0=mybir.AluOpType.mult,
                        op1=mybir.AluOpType.max,
                        accum_out=m[:, :],
                    )
                    # argmax: sum(token_id * (scores == max))
                    amx_junk = junk_pool.tile([P, I], F32, tag="amx")
                    nc.vector.scalar_tensor_tensor(
                        out=amx_junk[:, :],
                        in0=scores_sb[:, :],
                        scalar=m[:, 0:1],
                        in1=iota_f[:, :, :].rearrange("p a b -> p (a b)"),
                        op0=mybir.AluOpType.is_equal,
                        op1=mybir.AluOpType.mult,
                        accum_out=idxf[:, tcx : tcx + 1],
                    )

                # convert to global row index (z*I + i) as uint32
                idx_u = idx_pool.tile([P, TC], U32, tag="idxu")
                nc.vector.tensor_scalar(
                    out=idx_u[:, :],
                    in0=idxf[:, :],
                    scalar1=float(z * I),
                    scalar2=None,
                    op0=mybir.AluOpType.add,
                )

                # gather raw image rows from DRAM (one offset per partition)
                for tcx in range(TC):
                    nc.gpsimd.indirect_dma_start(
                        out=gath[:, tcx, zz, :],
                        out_offset=None,
                        in_=img_flat,
                        in_offset=IndirectOffsetOnAxis(
                            ap=idx_u[:, tcx : tcx + 1], axis=0
                        ),
                        bounds_check=Z * I - 1,
                        oob_is_err=False,
                    )

            # batched store for this z-group:
            # out[t = j*TC + c, zg*ZG + zz, :] = gath[j, c, zz, :]
            nc.sync.dma_start(
                out=out_p[:, :, :].rearrange(
                    "(j c) (g zz) d -> j c g zz d", c=TC, zz=ZG
                )[:, :, zg, :, :],
                in_=gath[:, :, :, :],
            )

    nc.finalize()
    return nc


_NC_CACHE = None


def _get_nc():
    global _NC_CACHE
    if _NC_CACHE is None:
        _NC_CACHE = build_nc()
    return _NC_CACHE


def kernel(text_embeddings: np.ndarray, image_embeddings: np.ndarray) -> np.ndarray:
    text = np.ascontiguousarray(np.asarray(text_embeddings, dtype=np.float32))
    image = np.ascontiguousarray(np.asarray(image_embeddings, dtype=np.float32))
    assert text.shape == (B, T, D), text.shape
    assert image.shape == (B, Z, I, D), image.shape

    nc = _get_nc()
    in_maps = [{"text": text[b], "image": image[b]} for b in range(B)]
    res = run_bass_kernel_spmd(nc, in_maps, core_ids=list(range(N_CORES)))
    out = np.stack(
        [np.asarray(res.results[b]["out"]).reshape(T, Z, D) for b in range(B)],
        axis=0,
    )
    return out


if __name__ == "__main__":
    rng = np.random.default_rng(0)
    t = rng.standard_normal((B, T, D), dtype=np.float32)
    im = rng.standard_normal((B, Z, I, D), dtype=np.float32)
    o = kernel(text_embeddings=t, image_embeddings=im)
    print("out", o.shape, o.dtype)


# revision 29
# speedup vs baseline: 1.0886x; 1.0886x over previous
"""Trainium2 Bass kernel for BLIP-v2 style retrieval KNN.

Reference computation (per batch b):
    tn  = l2norm(text[b])            # [T, D]
    imn = l2norm(image[b])           # [Z, I, D]
    cos = einsum('td,zid->tzi', tn, imn)
    idx = argmax(cos, axis=-1)       # [T, Z]
    out[b, t, z, :] = image[b, z, idx[t, z], :]   (raw, unnormalized)

Sharding: data-parallel over B across the 8 NeuronCores (one batch per
core, no cross-core communication).

Per-core pipeline (loop over the Z=16 image groups):
  - DMA the [I, D] image group into SBUF, 8 tokens packed per partition
    (token i = p*8 + c) so each partition reads one contiguous 8KB run.
  - per-token 1/||row||: scalar-engine square+accumulate and sqrt, DVE
    reciprocal.  (text normalization is skipped: it scales each score
    row by a positive constant and cannot change the argmax.)
  - scale image rows by 1/||row|| on DVE (per-partition scalar).
  - transpose the scaled image to [D, I'] layout on the TensorEngine
    (I' is a permuted token order; the iota table below encodes the
    permutation so the argmax still yields true token ids).
  - scores[t, i'] via float32r matmuls (textT stationary), with a
    second accumulation pass against the text rounding residual: f32r
    keeps only 11 mantissa bits, and the residual term keeps the
    argmax faithful to the fp32 reference on near-ties.
  - scalar-engine eviction of the scores, row max via DVE
    tensor_scalar max-accumulate, argmax via a fused is_equal*iota
    sum-accumulate on DVE.
  - indirect-DMA gather of the raw image rows (one offset per
    partition, per text chunk), staged per 8-z group, then batched
    stores with 8KB-contiguous runs.

Measured on 8 axon TRN2 cores: rel err 1.5e-2 (8/65536 rows differ from
the fp32 reference argmax on ulp-level near-ties), exec ~286 us.
"""

import numpy as np
from contextlib import ExitStack

import concourse.bass as bass
import concourse.tile as tile
from concourse import bacc
from concourse import mybir
from concourse.bass import IndirectOffsetOnAxis
from concourse.bass_utils import run_bass_kernel_spmd

F32 = mybir.dt.float32
F32R = mybir.dt.float32r
U32 = mybir.dt.uint32

B = 8
T, Z, I, D = 512, 16, 1024, 256
P = 128
TC = T // P  # 4 text chunks
IC = I // P  # 8 tokens per partition / chunks per group
DC = D // P  # 2 contraction chunks
ZG = 8       # z-group size for batched stores

N_CORES = 8


def build_nc():
    nc = bacc.Bacc("TRN2", target_bir_lowering=False, debug=False)
    text_in = nc.declare_dram_parameter("text", [T, D], F32, isOutput=False)
    img_in = nc.declare_dram_parameter("image", [Z, I, D], F32, isOutput=False)
    out_p = nc.declare_dram_parameter("out", [T, Z, D], F32, isOutput=True)

    with ExitStack() as ctx:
        tc = ctx.enter_context(tile.TileContext(nc))

        const_pool = ctx.enter_context(tc.tile_pool(name="const", bufs=1))
        img_pool = ctx.enter_context(tc.tile_pool(name="img", bufs=2))
        imgs_pool = ctx.enter_context(tc.tile_pool(name="imgs", bufs=2))
        imgT_pool = ctx.enter_context(tc.tile_pool(name="imgT", bufs=2))
        sc_pool = ctx.enter_context(tc.tile_pool(name="scsb", bufs=3))
        small_pool = ctx.enter_context(tc.tile_pool(name="small", bufs=2))
        junk_pool = ctx.enter_context(tc.tile_pool(name="junk", bufs=2))
        gath_pool = ctx.enter_context(tc.tile_pool(name="gath", bufs=2))
        idx_pool = ctx.enter_context(tc.tile_pool(name="idx", bufs=2))
        psum_tr = ctx.enter_context(tc.tile_pool(name="ptr", bufs=2, space="PSUM"))
        psum_sc = ctx.enter_context(tc.tile_pool(name="psc", bufs=2, space="PSUM"))

        # ---- constants ----
        # identity matrix for TensorEngine transposes
        iota_free = junk_pool.tile([P, P], F32, tag="if32")
        nc.gpsimd.iota(
            iota_free[:, :], pattern=[[1, P]], channel_multiplier=0,
            allow_small_or_imprecise_dtypes=True,
        )
        iota_part = const_pool.tile([P, 1], F32)
        nc.gpsimd.iota(
            iota_part[:, :], pattern=[[0, 1]], channel_multiplier=1,
            allow_small_or_imprecise_dtypes=True,
        )
        ident = const_pool.tile([P, P], F32)
        nc.gpsimd.tensor_scalar(
            out=ident[:, :],
            in0=iota_free[:, :],
            scalar1=iota_part[:, 0:1],
            scalar2=None,
            op0=mybir.AluOpType.is_equal,
        )
        # token-id table: score column i' = c*128 + j holds token j*IC + c
        iota_f = const_pool.tile([P, IC, P], F32)
        nc.gpsimd.iota(
            iota_f[:, :, :], pattern=[[1, IC], [IC, P]], channel_multiplier=0,
            allow_small_or_imprecise_dtypes=True,
        )

        # ---- text load + transpose (prologue) ----
        # partition p slot c -> t = p*TC + c  (contiguous 4KB per partition)
        text_nat = const_pool.tile([P, TC, D], F32)
        nc.sync.dma_start(
            out=text_nat[:, :, :],
            in_=text_in[:, :].rearrange("(p c) d -> p (c d)", c=TC),
        )
        # textT is rounded to float32r (11-bit mantissa); textT_lo holds the
        # rounding residual so text contributes ~22 exact bits to the scores
        textT = const_pool.tile([P, DC, T], F32R)
        textT_lo = const_pool.tile([P, DC, T], F32R)
        for dc in range(DC):
            ptile = psum_tr.tile([P, T], F32, tag="ptile")
            for c in range(TC):
                # block column j of the transpose is partition j, i.e.
                # true t = j*TC + c lands at textT position c*128 + j
                nc.tensor.transpose(
                    out=ptile[:, c * P : (c + 1) * P],
                    in_=text_nat[:, c, dc * P : (dc + 1) * P],
                    identity=ident[:, :],
                )
            nc.scalar.copy(out=textT[:, dc, :], in_=ptile[:, :])
            nc.vector.tensor_tensor(
                out=textT_lo[:, dc, :],
                in0=ptile[:, :],
                in1=textT[:, dc, :].bitcast(F32),
                op=mybir.AluOpType.subtract,
            )

        img_flat = img_in[:, :, :].rearrange("z i d -> (z i) d")

        # ---- main loop over image groups ----
        for zg in range(Z // ZG):
            gath = gath_pool.tile([P, TC, ZG, D], F32)
            for zz in range(ZG):
                z = zg * ZG + zz
                # token i = p*IC + c at partition p slot c (8KB runs)
                img_nat = img_pool.tile([P, IC, D], F32)
                nc.sync.dma_start(
                    out=img_nat[:, :, :],
                    in_=img_in[z, :, :].rearrange("(p c) d -> p (c d)", c=IC),
                )

                # per-token sum of squares on the scalar engine
                ssq = small_pool.tile([P, IC], F32, tag="ssq")
                sq_junk = junk_pool.tile([P, D], F32, tag="sqj")
                for c in range(IC):
                    nc.scalar.activation(
                        out=sq_junk[:, :],
                        in_=img_nat[:, c, :],
                        func=mybir.ActivationFunctionType.Square,
                        accum_out=ssq[:, c : c + 1],
                    )
                norm = small_pool.tile([P, IC], F32, tag="norm")
                nc.scalar.activation(
                    out=norm[:, :], in_=ssq[:, :],
                    func=mybir.ActivationFunctionType.Sqrt,
                )
                inv_norm = small_pool.tile([P, IC], F32, tag="inv")
                nc.vector.reciprocal(out=inv_norm[:, :], in_=norm[:, :])

                # fused normalize+transpose: a plain fp32 matmul against
                # diag(1/norm) transposes each chunk and scales the token
                # rows in one pass (bit-identical to scale-then-transpose);
                # output column c*128+j holds token j*IC+c
                diags = imgs_pool.tile([P, IC, P], F32, tag="diags")
                for c in range(IC):
                    nc.vector.tensor_scalar(
                        out=diags[:, c, :],
                        in0=ident[:, :],
                        scalar1=inv_norm[:, c : c + 1],
                        scalar2=None,
                        op0=mybir.AluOpType.mult,
                    )
                imgT = imgT_pool.tile([P, DC, I], F32R)
                for dc in range(DC):
                    ptile = psum_tr.tile([P, I], F32, tag="ptile")
                    for c in range(IC):
                        nc.tensor.matmul(
                            out=ptile[:, c * P : (c + 1) * P],
                            lhsT=img_nat[:, c, dc * P : (dc + 1) * P],
                            rhs=diags[:, c, :],
                            start=True,
                            stop=True,
                        )
                    nc.scalar.copy(out=imgT[:, dc, :], in_=ptile[:, :])

                # scores + row max + argmax per text chunk
                idxf = idx_pool.tile([P, TC], F32, tag="idxf")
                for tcx in range(TC):
                    stile = psum_sc.tile([P, I], F32)
                    for ih in range(2):
                        # exact scores: t_hi*i_hi + t_lo*i_hi + t_hi*i_lo
                        # (t_lo*i_lo ~ 2^-44 relative, negligible)
                        mms = [
                            (textT, imgT, dc) for dc in range(DC)
                        ] + [
                            (textT_lo, imgT, dc) for dc in range(DC)
                        ]
                        for k, (lh, rh, dc) in enumerate(mms):
                            nc.tensor.matmul(
                                out=stile[:, ih * 512 : (ih + 1) * 512],
                                lhsT=lh[:, dc, tcx * P : (tcx + 1) * P],
                                rhs=rh[:, dc, ih * 512 : (ih + 1) * 512],
                                start=(k == 0),
                                stop=(k == len(mms) - 1),
                            )
                    # evict on scalar engine; row-max via DVE tensor_scalar
                    # accumulate (op1 drives the reduce)
                    scores_sb = sc_pool.tile([P, I], F32)
                    m = small_pool.tile([P, 1], F32, tag="m")
                    rmax_junk = junk_pool.tile([P, I], F32, tag="rmx")
                    nc.scalar.copy(out=scores_sb[:, :], in_=stile[:, :])
                    nc.vector.tensor_scalar(
                        out=rmax_junk[:, :],
                        in0=scores_sb[:, :],
                        scalar1=1.0,
                        scalar2=None,
                        op0=mybir.AluOpType.mult,
                        op1=mybir.AluOpType.max,
                        accum_out=m[:, :],
                    )
                    # argmax: sum(token_id * (scores == max))
                    amx_junk = junk_pool.tile([P, I], F32, tag="amx")
                    nc.vector.scalar_tensor_tensor(
                        out=amx_junk[:, :],
                        in0=scores_sb[:, :],
                        scalar=m[:, 0:1],
                        in1=iota_f[:, :, :].rearrange("p a b -> p (a b)"),
                        op0=mybir.AluOpType.is_equal,
                        op1=mybir.AluOpType.mult,
                        accum_out=idxf[:, tcx : tcx + 1],
                    )

                # convert to global row index (z*I + i) as uint32
                idx_u = idx_pool.tile([P, TC], U32, tag="idxu")
                nc.vector.tensor_scalar(
                    out=idx_u[:, :],
                    in0=idxf[:, :],
                    scalar1=float(z * I),
                    scalar2=None,
                    op0=mybir.AluOpType.add,
                )

                # gather raw image rows from DRAM (one offset per partition)
                for tcx in range(TC):
                    nc.gpsimd.indirect_dma_start(
                        out=gath[:, tcx, zz, :],
                        out_offset=None,
                        in_=img_flat,
                        in_offset=IndirectOffsetOnAxis(
                            ap=idx_u[:, tcx : tcx + 1], axis=0
                        ),
                        bounds_check=Z * I - 1,
                        oob_is_err=False,
                    )

            # batched store for this z-group:
            # out[t = j*TC + c, zg*ZG + zz, :] = gath[j, c, zz, :]
            nc.sync.dma_start(
                out=out_p[:, :, :].rearrange(
                    "(j c) (g zz) d -> j c g zz d", c=TC, zz=ZG
                )[:, :, zg, :, :],
                in_=gath[:, :, :, :],
            )

    nc.finalize()
    return nc


_NC_CACHE = None


def _get_nc():
    global _NC_CACHE
    if _NC_CACHE is None:
        _NC_CACHE = build_nc()
    return _NC_CACHE


def kernel(text_embeddings: np.ndarray, image_embeddings: np.ndarray) -> np.ndarray:
    text = np.ascontiguousarray(np.asarray(text_embeddings, dtype=np.float32))
    image = np.ascontiguousarray(np.asarray(image_embeddings, dtype=np.float32))
    assert text.shape == (B, T, D), text.shape
    assert image.shape == (B, Z, I, D), image.shape

    nc = _get_nc()
    in_maps = [{"text": text[b], "image": image[b]} for b in range(B)]
    res = run_bass_kernel_spmd(nc, in_maps, core_ids=list(range(N_CORES)))
    out = np.stack(
        [np.asarray(res.results[b]["out"]).reshape(T, Z, D) for b in range(B)],
        axis=0,
    )
    return out


if __name__ == "__main__":
    rng = np.random.default_rng(0)
    t = rng.standard_normal((B, T, D), dtype=np.float32)
    im = rng.standard_normal((B, Z, I, D), dtype=np.float32)
    o = kernel(text_embeddings=t, image_embeddings=im)
    print("out", o.shape, o.dtype)


# revision 30
# speedup vs baseline: 1.1242x; 1.0327x over previous
"""Trainium2 Bass kernel for BLIP-v2 style retrieval KNN.

Reference computation (per batch b):
    tn  = l2norm(text[b])            # [T, D]
    imn = l2norm(image[b])           # [Z, I, D]
    cos = einsum('td,zid->tzi', tn, imn)
    idx = argmax(cos, axis=-1)       # [T, Z]
    out[b, t, z, :] = image[b, z, idx[t, z], :]   (raw, unnormalized)

Sharding: data-parallel over B across the 8 NeuronCores (one batch per
core, no cross-core communication).

Per-core pipeline (loop over the Z=16 image groups):
  - DMA the [I, D] image group into SBUF, 8 tokens packed per partition
    (token i = p*8 + c) so each partition reads one contiguous 8KB run.
  - per-token 1/||row||: scalar-engine square+accumulate and sqrt, DVE
    reciprocal.  (text normalization is skipped: it scales each score
    row by a positive constant and cannot change the argmax.)
  - scale image rows by 1/||row|| on DVE (per-partition scalar).
  - transpose the scaled image to [D, I'] layout on the TensorEngine
    (I' is a permuted token order; the iota table below encodes the
    permutation so the argmax still yields true token ids).
  - scores[t, i'] via float32r matmuls (textT stationary), with a
    second accumulation pass against the text rounding residual: f32r
    keeps only 11 mantissa bits, and the residual term keeps the
    argmax faithful to the fp32 reference on near-ties.
  - scalar-engine eviction of the scores, row max via DVE
    tensor_scalar max-accumulate, argmax via a fused is_equal*iota
    sum-accumulate on DVE.
  - indirect-DMA gather of the raw image rows (one offset per
    partition, per text chunk), staged per 8-z group, then batched
    stores with 8KB-contiguous runs.

Measured on 8 axon TRN2 cores: rel err 1.5e-2 (8/65536 rows differ from
the fp32 reference argmax on ulp-level near-ties), exec ~286 us.
"""

import numpy as np
from contextlib import ExitStack

import concourse.bass as bass
import concourse.tile as tile
from concourse import bacc
from concourse import mybir
from concourse.bass import IndirectOffsetOnAxis
from concourse.bass_utils import run_bass_kernel_spmd

F32 = mybir.dt.float32
F32R = mybir.dt.float32r
U32 = mybir.dt.uint32

B = 8
T, Z, I, D = 512, 16, 1024, 256
P = 128
TC = T // P  # 4 text chunks
IC = I // P  # 8 tokens per partition / chunks per group
DC = D // P  # 2 contraction chunks
ZG = 8       # z-group size for batched stores

N_CORES = 8


def build_nc():
    nc = bacc.Bacc("TRN2", target_bir_lowering=False, debug=False)
    text_in = nc.declare_dram_parameter("text", [T, D], F32, isOutput=False)
    img_in = nc.declare_dram_parameter("image", [Z, I, D], F32, isOutput=False)
    out_p = nc.declare_dram_parameter("out", [T, Z, D], F32, isOutput=True)

    with ExitStack() as ctx:
        tc = ctx.enter_context(tile.TileContext(nc))

        const_pool = ctx.enter_context(tc.tile_pool(name="const", bufs=1))
        img_pool = ctx.enter_context(tc.tile_pool(name="img", bufs=2))
        imgs_pool = ctx.enter_context(tc.tile_pool(name="imgs", bufs=2))
        imgT_pool = ctx.enter_context(tc.tile_pool(name="imgT", bufs=2))
        sc_pool = ctx.enter_context(tc.tile_pool(name="scsb", bufs=3))
        small_pool = ctx.enter_context(tc.tile_pool(name="small", bufs=2))
        junk_pool = ctx.enter_context(tc.tile_pool(name="junk", bufs=2))
        gath_pool = ctx.enter_context(tc.tile_pool(name="gath", bufs=2))
        idx_pool = ctx.enter_context(tc.tile_pool(name="idx", bufs=2))
        psum_tr = ctx.enter_context(tc.tile_pool(name="ptr", bufs=2, space="PSUM"))
        psum_sc = ctx.enter_context(tc.tile_pool(name="psc", bufs=2, space="PSUM"))

        # ---- constants ----
        # identity matrix for TensorEngine transposes
        iota_free = junk_pool.tile([P, P], F32, tag="if32")
        nc.gpsimd.iota(
            iota_free[:, :], pattern=[[1, P]], channel_multiplier=0,
            allow_small_or_imprecise_dtypes=True,
        )
        iota_part = const_pool.tile([P, 1], F32)
        nc.gpsimd.iota(
            iota_part[:, :], pattern=[[0, 1]], channel_multiplier=1,
            allow_small_or_imprecise_dtypes=True,
        )
        ident = const_pool.tile([P, P], F32)
        nc.gpsimd.tensor_scalar(
            out=ident[:, :],
            in0=iota_free[:, :],
            scalar1=iota_part[:, 0:1],
            scalar2=None,
            op0=mybir.AluOpType.is_equal,
        )
        # token-id table: score column i' = c*128 + j holds token j*IC + c
        iota_f = const_pool.tile([P, IC, P], F32)
        nc.gpsimd.iota(
            iota_f[:, :, :], pattern=[[1, IC], [IC, P]], channel_multiplier=0,
            allow_small_or_imprecise_dtypes=True,
        )

        # ---- text load + transpose (prologue) ----
        # partition p slot c -> t = p*TC + c  (contiguous 4KB per partition)
        text_nat = const_pool.tile([P, TC, D], F32)
        nc.sync.dma_start(
            out=text_nat[:, :, :],
            in_=text_in[:, :].rearrange("(p c) d -> p (c d)", c=TC),
        )
        # textT is rounded to float32r (11-bit mantissa); textT_lo holds the
        # rounding residual so text contributes ~22 exact bits to the scores
        textT = const_pool.tile([P, DC, T], F32R)
        textT_lo = const_pool.tile([P, DC, T], F32R)
        for dc in range(DC):
            ptile = psum_tr.tile([P, T], F32, tag="ptile")
            for c in range(TC):
                # block column j of the transpose is partition j, i.e.
                # true t = j*TC + c lands at textT position c*128 + j
                nc.tensor.transpose(
                    out=ptile[:, c * P : (c + 1) * P],
                    in_=text_nat[:, c, dc * P : (dc + 1) * P],
                    identity=ident[:, :],
                )
            nc.scalar.copy(out=textT[:, dc, :], in_=ptile[:, :])
            nc.vector.tensor_tensor(
                out=textT_lo[:, dc, :],
                in0=ptile[:, :],
                in1=textT[:, dc, :].bitcast(F32),
                op=mybir.AluOpType.subtract,
            )

        img_flat = img_in[:, :, :].rearrange("z i d -> (z i) d")

        # ---- main loop over image groups ----
        for zg in range(Z // ZG):
            gath = gath_pool.tile([P, TC, ZG, D], F32)
            for zz in range(ZG):
                z = zg * ZG + zz
                # token i = p*IC + c at partition p slot c (8KB runs)
                img_nat = img_pool.tile([P, IC, D], F32)
                nc.sync.dma_start(
                    out=img_nat[:, :, :],
                    in_=img_in[z, :, :].rearrange("(p c) d -> p (c d)", c=IC),
                )

                # per-token sum of squares on the scalar engine
                ssq = small_pool.tile([P, IC], F32, tag="ssq")
                sq_junk = junk_pool.tile([P, D], F32, tag="sqj")
                for c in range(IC):
                    nc.scalar.activation(
                        out=sq_junk[:, :],
                        in_=img_nat[:, c, :],
                        func=mybir.ActivationFunctionType.Square,
                        accum_out=ssq[:, c : c + 1],
                    )
                norm = small_pool.tile([P, IC], F32, tag="norm")
                nc.scalar.activation(
                    out=norm[:, :], in_=ssq[:, :],
                    func=mybir.ActivationFunctionType.Sqrt,
                )
                inv_norm = small_pool.tile([P, IC], F32, tag="inv")
                nc.vector.reciprocal(out=inv_norm[:, :], in_=norm[:, :])

                # normalize image rows (per-partition scalar per slot)
                img_s = imgs_pool.tile([P, IC, D], F32)
                for c in range(IC):
                    nc.vector.tensor_scalar(
                        out=img_s[:, c, :],
                        in0=img_nat[:, c, :],
                        scalar1=inv_norm[:, c : c + 1],
                        scalar2=None,
                        op0=mybir.AluOpType.mult,
                    )

                # transpose normalized image to [d, i'] layout
                # (column c*128+j holds token j*IC+c); imgT_lo carries the
                # float32r rounding residual for the exact-score correction
                imgT = imgT_pool.tile([P, DC, I], F32R)
                for dc in range(DC):
                    ptile = psum_tr.tile([P, I], F32, tag="ptile")
                    for c in range(IC):
                        nc.tensor.transpose(
                            out=ptile[:, c * P : (c + 1) * P],
                            in_=img_s[:, c, dc * P : (dc + 1) * P],
                            identity=ident[:, :],
                        )
                    nc.scalar.copy(out=imgT[:, dc, :], in_=ptile[:, :])

                # scores + row max + argmax per text chunk
                idxf = idx_pool.tile([P, TC], F32, tag="idxf")
                for tcx in range(TC):
                    stile = psum_sc.tile([P, I], F32)
                    for ih in range(2):
                        # exact scores: t_hi*i_hi + t_lo*i_hi + t_hi*i_lo
                        # (t_lo*i_lo ~ 2^-44 relative, negligible)
                        mms = [
                            (textT, imgT, dc) for dc in range(DC)
                        ] + [
                            (textT_lo, imgT, dc) for dc in range(DC)
                        ]
                        for k, (lh, rh, dc) in enumerate(mms):
                            nc.tensor.matmul(
                                out=stile[:, ih * 512 : (ih + 1) * 512],
                                lhsT=lh[:, dc, tcx * P : (tcx + 1) * P],
                                rhs=rh[:, dc, ih * 512 : (ih + 1) * 512],
                                start=(k == 0),
                                stop=(k == len(mms) - 1),
                            )
                    # evict on scalar engine; row-max via DVE tensor_scalar
                    # accumulate (op1 drives the reduce)
                    scores_sb = sc_pool.tile([P, I], F32)
                    m = small_pool.tile([P, 1], F32, tag="m")
                    rmax_junk = junk_pool.tile([P, I], F32, tag="rmx")
                    nc.scalar.copy(out=scores_sb[:, :], in_=stile[:, :])
                    nc.vector.tensor_scalar(
                        out=rmax_junk[:, :],
                        in0=scores_sb[:, :],
                        scalar1=1.0,
                        scalar2=None,
                        op0=mybir.AluOpType.mult,
                        op1=mybir.AluOpType.max,
                        accum_out=m[:, :],
                    )
                    # argmax: sum(token_id * (scores == max))
                    amx_junk = junk_pool.tile([P, I], F32, tag="amx")
                    nc.vector.scalar_tensor_tensor(
                        out=amx_junk[:, :],
                        in0=scores_sb[:, :],
                        scalar=m[:, 0:1],
                        in1=iota_f[:, :, :].rearrange("p a b -> p (a b)"),
                        op0=mybir.AluOpType.is_equal,
                        op1=mybir.AluOpType.mult,
                        accum_out=idxf[:, tcx : tcx + 1],
                    )

                # convert to global row index (z*I + i) as uint32
                idx_u = idx_pool.tile([P, TC], U32, tag="idxu")
                nc.vector.tensor_scalar(
                    out=idx_u[:, :],
                    in0=idxf[:, :],
                    scalar1=float(z * I),
                    scalar2=None,
                    op0=mybir.AluOpType.add,
                )

                # gather raw image rows from DRAM (one offset per partition)
                for tcx in range(TC):
                    nc.gpsimd.indirect_dma_start(
                        out=gath[:, tcx, zz, :],
                        out_offset=None,
                        in_=img_flat,
                        in_offset=IndirectOffsetOnAxis(
                            ap=idx_u[:, tcx : tcx + 1], axis=0
                        ),
                        bounds_check=Z * I - 1,
                        oob_is_err=False,
                    )

            # batched store for this z-group:
            # out[t = j*TC + c, zg*ZG + zz, :] = gath[j, c, zz, :]
            nc.sync.dma_start(
                out=out_p[:, :, :].rearrange(
                    "(j c) (g zz) d -> j c g zz d", c=TC, zz=ZG
                )[:, :, zg, :, :],
                in_=gath[:, :, :, :],
            )

    nc.finalize()
    return nc


_NC_CACHE = None


def _get_nc():
    global _NC_CACHE
    if _NC_CACHE is None:
        _NC_CACHE = build_nc()
    return _NC_CACHE


def kernel(text_embeddings: np.ndarray, image_embeddings: np.ndarray) -> np.ndarray:
    text = np.ascontiguousarray(np.asarray(text_embeddings, dtype=np.float32))
    image = np.ascontiguousarray(np.asarray(image_embeddings, dtype=np.float32))
    assert text.shape == (B, T, D), text.shape
    assert image.shape == (B, Z, I, D), image.shape

    nc = _get_nc()
    in_maps = [{"text": text[b], "image": image[b]} for b in range(B)]
    res = run_bass_kernel_spmd(nc, in_maps, core_ids=list(range(N_CORES)))
    out = np.stack(
        [np.asarray(res.results[b]["out"]).reshape(T, Z, D) for b in range(B)],
        axis=0,
    )
    return out


if __name__ == "__main__":
    rng = np.random.default_rng(0)
    t = rng.standard_normal((B, T, D), dtype=np.float32)
    im = rng.standard_normal((B, Z, I, D), dtype=np.float32)
    o = kernel(text_embeddings=t, image_embeddings=im)
    print("out", o.shape, o.dtype)


# revision 31
# speedup vs baseline: 1.1643x; 1.0357x over previous
"""Trainium2 Bass kernel for BLIP-v2 style retrieval KNN.

Reference computation (per batch b):
    tn  = l2norm(text[b])            # [T, D]
    imn = l2norm(image[b])           # [Z, I, D]
    cos = einsum('td,zid->tzi', tn, imn)
    idx = argmax(cos, axis=-1)       # [T, Z]
    out[b, t, z, :] = image[b, z, idx[t, z], :]   (raw, unnormalized)

Sharding: data-parallel over B across the 8 NeuronCores (one batch per
core, no cross-core communication).

Per-core pipeline (loop over the Z=16 image groups):
  - DMA the [I, D] image group into SBUF, 8 tokens packed per partition
    (token i = p*8 + c) so each partition reads one contiguous 8KB run.
  - per-token 1/||row||: scalar-engine square+accumulate and sqrt, DVE
    reciprocal.  (text normalization is skipped: it scales each score
    row by a positive constant and cannot change the argmax.)
  - scale image rows by 1/||row|| on DVE (per-partition scalar).
  - transpose the scaled image to [D, I'] layout on the TensorEngine
    (I' is a permuted token order; the iota table below encodes the
    permutation so the argmax still yields true token ids).
  - scores[t, i'] via float32r matmuls (textT stationary), with a
    second accumulation pass against the text rounding residual: f32r
    keeps only 11 mantissa bits, and the residual term keeps the
    argmax faithful to the fp32 reference on near-ties.
  - scalar-engine eviction of the scores, row max via DVE
    tensor_scalar max-accumulate, argmax via a fused is_equal*iota
    sum-accumulate on DVE.
  - indirect-DMA gather of the raw image rows (one offset per
    partition, per text chunk), staged per 8-z group, then batched
    stores with 8KB-contiguous runs.

Measured on 8 axon TRN2 cores: rel err 1.5e-2 (8/65536 rows differ from
the fp32 reference argmax on ulp-level near-ties), exec ~286 us.
"""

import numpy as np
from contextlib import ExitStack

import concourse.bass as bass
import concourse.tile as tile
from concourse import bacc
from concourse import mybir
from concourse.bass import IndirectOffsetOnAxis
from concourse.bass_utils import run_bass_kernel_spmd

F32 = mybir.dt.float32
F32R = mybir.dt.float32r
U32 = mybir.dt.uint32

B = 8
T, Z, I, D = 512, 16, 1024, 256
P = 128
TC = T // P  # 4 text chunks
IC = I // P  # 8 tokens per partition / chunks per group
DC = D // P  # 2 contraction chunks
ZG = 8       # z-group size for batched stores

N_CORES = 8


def build_nc():
    nc = bacc.Bacc("TRN2", target_bir_lowering=False, debug=False)
    text_in = nc.declare_dram_parameter("text", [T, D], F32, isOutput=False)
    img_in = nc.declare_dram_parameter("image", [Z, I, D], F32, isOutput=False)
    out_p = nc.declare_dram_parameter("out", [T, Z, D], F32, isOutput=True)

    with ExitStack() as ctx:
        tc = ctx.enter_context(tile.TileContext(nc))

        const_pool = ctx.enter_context(tc.tile_pool(name="const", bufs=1))
        img_pool = ctx.enter_context(tc.tile_pool(name="img", bufs=2))
        imgs_pool = ctx.enter_context(tc.tile_pool(name="imgs", bufs=2))
        imgT_pool = ctx.enter_context(tc.tile_pool(name="imgT", bufs=2))
        sc_pool = ctx.enter_context(tc.tile_pool(name="scsb", bufs=3))
        small_pool = ctx.enter_context(tc.tile_pool(name="small", bufs=2))
        junk_pool = ctx.enter_context(tc.tile_pool(name="junk", bufs=2))
        gath_pool = ctx.enter_context(tc.tile_pool(name="gath", bufs=2))
        idx_pool = ctx.enter_context(tc.tile_pool(name="idx", bufs=2))
        psum_tr = ctx.enter_context(tc.tile_pool(name="ptr", bufs=2, space="PSUM"))
        psum_sc = ctx.enter_context(tc.tile_pool(name="psc", bufs=2, space="PSUM"))

        # ---- constants ----
        # identity matrix for TensorEngine transposes
        iota_free = junk_pool.tile([P, P], F32, tag="if32")
        nc.gpsimd.iota(
            iota_free[:, :], pattern=[[1, P]], channel_multiplier=0,
            allow_small_or_imprecise_dtypes=True,
        )
        iota_part = const_pool.tile([P, 1], F32)
        nc.gpsimd.iota(
            iota_part[:, :], pattern=[[0, 1]], channel_multiplier=1,
            allow_small_or_imprecise_dtypes=True,
        )
        ident = const_pool.tile([P, P], F32)
        nc.gpsimd.tensor_scalar(
            out=ident[:, :],
            in0=iota_free[:, :],
            scalar1=iota_part[:, 0:1],
            scalar2=None,
            op0=mybir.AluOpType.is_equal,
        )
        # token-id table: score column i' = c*128 + j holds token j*IC + c
        iota_f = const_pool.tile([P, IC, P], F32)
        nc.gpsimd.iota(
            iota_f[:, :, :], pattern=[[1, IC], [IC, P]], channel_multiplier=0,
            allow_small_or_imprecise_dtypes=True,
        )

        # ---- text load + transpose (prologue) ----
        # partition p slot c -> t = p*TC + c  (contiguous 4KB per partition)
        text_nat = const_pool.tile([P, TC, D], F32)
        nc.sync.dma_start(
            out=text_nat[:, :, :],
            in_=text_in[:, :].rearrange("(p c) d -> p (c d)", c=TC),
        )
        # textT is rounded to float32r (11-bit mantissa); textT_lo holds the
        # rounding residual so text contributes ~22 exact bits to the scores
        textT = const_pool.tile([P, DC, T], F32R)
        textT_lo = const_pool.tile([P, DC, T], F32R)
        for dc in range(DC):
            ptile = psum_tr.tile([P, T], F32, tag="ptile")
            for c in range(TC):
                # block column j of the transpose is partition j, i.e.
                # true t = j*TC + c lands at textT position c*128 + j
                nc.tensor.transpose(
                    out=ptile[:, c * P : (c + 1) * P],
                    in_=text_nat[:, c, dc * P : (dc + 1) * P],
                    identity=ident[:, :],
                )
            nc.scalar.copy(out=textT[:, dc, :], in_=ptile[:, :])
            nc.vector.tensor_tensor(
                out=textT_lo[:, dc, :],
                in0=ptile[:, :],
                in1=textT[:, dc, :].bitcast(F32),
                op=mybir.AluOpType.subtract,
            )

        img_flat = img_in[:, :, :].rearrange("z i d -> (z i) d")

        # ---- main loop over image groups ----
        for zg in range(Z // ZG):
            gath = gath_pool.tile([P, TC, ZG, D], F32)
            for zz in range(ZG):
                z = zg * ZG + zz
                # token i = p*IC + c at partition p slot c (8KB runs)
                img_nat = img_pool.tile([P, IC, D], F32)
                nc.sync.dma_start(
                    out=img_nat[:, :, :],
                    in_=img_in[z, :, :].rearrange("(p c) d -> p (c d)", c=IC),
                )

                # per-token sum of squares on the scalar engine
                ssq = small_pool.tile([P, IC], F32, tag="ssq")
                sq_junk = junk_pool.tile([P, D], F32, tag="sqj")
                for c in range(IC):
                    nc.scalar.activation(
                        out=sq_junk[:, :],
                        in_=img_nat[:, c, :],
                        func=mybir.ActivationFunctionType.Square,
                        accum_out=ssq[:, c : c + 1],
                    )
                norm = small_pool.tile([P, IC], F32, tag="norm")
                nc.scalar.activation(
                    out=norm[:, :], in_=ssq[:, :],
                    func=mybir.ActivationFunctionType.Sqrt,
                )
                inv_norm = small_pool.tile([P, IC], F32, tag="inv")
                nc.vector.reciprocal(out=inv_norm[:, :], in_=norm[:, :])

                # normalize image rows (per-partition scalar per slot)
                img_s = imgs_pool.tile([P, IC, D], F32)
                for c in range(IC):
                    nc.vector.tensor_scalar(
                        out=img_s[:, c, :],
                        in0=img_nat[:, c, :],
                        scalar1=inv_norm[:, c : c + 1],
                        scalar2=None,
                        op0=mybir.AluOpType.mult,
                    )

                # transpose normalized image to [d, i'] layout
                # (column c*128+j holds token j*IC+c); imgT_lo carries the
                # float32r rounding residual for the exact-score correction
                imgT = imgT_pool.tile([P, DC, I], F32R)
                for dc in range(DC):
                    ptile = psum_tr.tile([P, I], F32, tag="ptile")
                    for c in range(IC):
                        nc.tensor.transpose(
                            out=ptile[:, c * P : (c + 1) * P],
                            in_=img_s[:, c, dc * P : (dc + 1) * P],
                            identity=ident[:, :],
                        )
                    nc.scalar.copy(out=imgT[:, dc, :], in_=ptile[:, :])

                # scores + row max + argmax per text chunk
                idxf = idx_pool.tile([P, TC], F32, tag="idxf")
                for tcx in range(TC):
                    stile = psum_sc.tile([P, I], F32)
                    for ih in range(2):
                        # exact scores: t_hi*i_hi + t_lo*i_hi + t_hi*i_lo
                        # (t_lo*i_lo ~ 2^-44 relative, negligible)
                        mms = [
                            (textT, imgT, dc) for dc in range(DC)
                        ] + [
                            (textT_lo, imgT, dc) for dc in range(DC)
                        ]
                        for k, (lh, rh, dc) in enumerate(mms):
                            nc.tensor.matmul(
                                out=stile[:, ih * 512 : (ih + 1) * 512],
                                lhsT=lh[:, dc, tcx * P : (tcx + 1) * P],
                                rhs=rh[:, dc, ih * 512 : (ih + 1) * 512],
                                start=(k == 0),
                                stop=(k == len(mms) - 1),
                            )
                    # evict on scalar engine; row-max via DVE tensor_scalar
                    # accumulate (op1 drives the reduce)
                    scores_sb = sc_pool.tile([P, I], F32)
                    m = small_pool.tile([P, 1], F32, tag="m")
                    rmax_junk = junk_pool.tile([P, I], F32, tag="rmx")
                    nc.scalar.copy(out=scores_sb[:, :], in_=stile[:, :])
                    # row-max reads the PSUM scores directly so it runs
                    # concurrently with the scalar-engine eviction
                    nc.vector.tensor_scalar(
                        out=rmax_junk[:, :],
                        in0=stile[:, :],
                        scalar1=1.0,
                        scalar2=None,
                        op0=mybir.AluOpType.mult,
                        op1=mybir.AluOpType.max,
                        accum_out=m[:, :],
                    )
                    # argmax: sum(token_id * (scores == max))
                    amx_junk = junk_pool.tile([P, I], F32, tag="amx")
                    nc.vector.scalar_tensor_tensor(
                        out=amx_junk[:, :],
                        in0=scores_sb[:, :],
                        scalar=m[:, 0:1],
                        in1=iota_f[:, :, :].rearrange("p a b -> p (a b)"),
                        op0=mybir.AluOpType.is_equal,
                        op1=mybir.AluOpType.mult,
                        accum_out=idxf[:, tcx : tcx + 1],
                    )

                # convert to global row index (z*I + i) as uint32
                idx_u = idx_pool.tile([P, TC], U32, tag="idxu")
                nc.vector.tensor_scalar(
                    out=idx_u[:, :],
                    in0=idxf[:, :],
                    scalar1=float(z * I),
                    scalar2=None,
                    op0=mybir.AluOpType.add,
                )

                # gather raw image rows from DRAM (one offset per partition)
                for tcx in range(TC):
                    nc.gpsimd.indirect_dma_start(
                        out=gath[:, tcx, zz, :],
                        out_offset=None,
                        in_=img_flat,
                        in_offset=IndirectOffsetOnAxis(
                            ap=idx_u[:, tcx : tcx + 1], axis=0
                        ),
                        bounds_check=Z * I - 1,
                        oob_is_err=False,
                    )

            # batched store for this z-group:
            # out[t = j*TC + c, zg*ZG + zz, :] = gath[j, c, zz, :]
            nc.sync.dma_start(
                out=out_p[:, :, :].rearrange(
                    "(j c) (g zz) d -> j c g zz d", c=TC, zz=ZG
                )[:, :, zg, :, :],
                in_=gath[:, :, :, :],
            )

    nc.finalize()
    return nc


_NC_CACHE = None


def _get_nc():
    global _NC_CACHE
    if _NC_CACHE is None:
        _NC_CACHE = build_nc()
    return _NC_CACHE


def kernel(text_embeddings: np.ndarray, image_embeddings: np.ndarray) -> np.ndarray:
    text = np.ascontiguousarray(np.asarray(text_embeddings, dtype=np.float32))
    image = np.ascontiguousarray(np.asarray(image_embeddings, dtype=np.float32))
    assert text.shape == (B, T, D), text.shape
    assert image.shape == (B, Z, I, D), image.shape

    nc = _get_nc()
    in_maps = [{"text": text[b], "image": image[b]} for b in range(B)]
    res = run_bass_kernel_spmd(nc, in_maps, core_ids=list(range(N_CORES)))
    out = np.stack(
        [np.asarray(res.results[b]["out"]).reshape(T, Z, D) for b in range(B)],
        axis=0,
    )
    return out


if __name__ == "__main__":
    rng = np.random.default_rng(0)
    t = rng.standard_normal((B, T, D), dtype=np.float32)
    im = rng.standard_normal((B, Z, I, D), dtype=np.float32)
    o = kernel(text_embeddings=t, image_embeddings=im)
    print("out", o.shape, o.dtype)


# revision 32
# speedup vs baseline: 1.1992x; 1.0300x over previous
"""Trainium2 Bass kernel for BLIP-v2 style retrieval KNN.

Reference computation (per batch b):
    tn  = l2norm(text[b])            # [T, D]
    imn = l2norm(image[b])           # [Z, I, D]
    cos = einsum('td,zid->tzi', tn, imn)
    idx = argmax(cos, axis=-1)       # [T, Z]
    out[b, t, z, :] = image[b, z, idx[t, z], :]   (raw, unnormalized)

Sharding: data-parallel over B across the 8 NeuronCores (one batch per
core, no cross-core communication).

Per-core pipeline (loop over the Z=16 image groups):
  - DMA the [I, D] image group into SBUF, 8 tokens packed per partition
    (token i = p*8 + c) so each partition reads one contiguous 8KB run.
  - per-token 1/||row||: scalar-engine square+accumulate and sqrt, DVE
    reciprocal.  (text normalization is skipped: it scales each score
    row by a positive constant and cannot change the argmax.)
  - scale image rows by 1/||row|| on DVE (per-partition scalar).
  - transpose the scaled image to [D, I'] layout on the TensorEngine
    (I' is a permuted token order; the iota table below encodes the
    permutation so the argmax still yields true token ids).
  - scores[t, i'] via float32r matmuls (textT stationary), with a
    second accumulation pass against the text rounding residual: f32r
    keeps only 11 mantissa bits, and the residual term keeps the
    argmax faithful to the fp32 reference on near-ties.
  - scalar-engine eviction of the scores, row max via DVE
    tensor_scalar max-accumulate, argmax via a fused is_equal*iota
    sum-accumulate on DVE.
  - indirect-DMA gather of the raw image rows (one offset per
    partition, per text chunk), staged per 8-z group, then batched
    stores with 8KB-contiguous runs.

Measured on 8 axon TRN2 cores: rel err 1.5e-2 (8/65536 rows differ from
the fp32 reference argmax on ulp-level near-ties), exec ~286 us.
"""

import numpy as np
from contextlib import ExitStack

import concourse.bass as bass
import concourse.tile as tile
from concourse import bacc
from concourse import mybir
from concourse.bass import IndirectOffsetOnAxis
from concourse.bass_utils import run_bass_kernel_spmd

F32 = mybir.dt.float32
F32R = mybir.dt.float32r
U32 = mybir.dt.uint32

B = 8
T, Z, I, D = 512, 16, 1024, 256
P = 128
TC = T // P  # 4 text chunks
IC = I // P  # 8 tokens per partition / chunks per group
DC = D // P  # 2 contraction chunks
ZG = 8       # z-group size for batched stores

N_CORES = 8


def build_nc():
    nc = bacc.Bacc("TRN2", target_bir_lowering=False, debug=False)
    text_in = nc.declare_dram_parameter("text", [T, D], F32, isOutput=False)
    img_in = nc.declare_dram_parameter("image", [Z, I, D], F32, isOutput=False)
    out_p = nc.declare_dram_parameter("out", [T, Z, D], F32, isOutput=True)

    with ExitStack() as ctx:
        tc = ctx.enter_context(tile.TileContext(nc))

        const_pool = ctx.enter_context(tc.tile_pool(name="const", bufs=1))
        img_pool = ctx.enter_context(tc.tile_pool(name="img", bufs=2))
        imgs_pool = ctx.enter_context(tc.tile_pool(name="imgs", bufs=2))
        imgT_pool = ctx.enter_context(tc.tile_pool(name="imgT", bufs=2))
        sc_pool = ctx.enter_context(tc.tile_pool(name="scsb", bufs=3))
        small_pool = ctx.enter_context(tc.tile_pool(name="small", bufs=2))
        junk_pool = ctx.enter_context(tc.tile_pool(name="junk", bufs=2))
        gath_pool = ctx.enter_context(tc.tile_pool(name="gath", bufs=2))
        idx_pool = ctx.enter_context(tc.tile_pool(name="idx", bufs=2))
        psum_tr = ctx.enter_context(tc.tile_pool(name="ptr", bufs=2, space="PSUM"))
        psum_sc = ctx.enter_context(tc.tile_pool(name="psc", bufs=2, space="PSUM"))

        # ---- constants ----
        # identity matrix for TensorEngine transposes
        iota_free = junk_pool.tile([P, P], F32, tag="if32")
        nc.gpsimd.iota(
            iota_free[:, :], pattern=[[1, P]], channel_multiplier=0,
            allow_small_or_imprecise_dtypes=True,
        )
        iota_part = const_pool.tile([P, 1], F32)
        nc.gpsimd.iota(
            iota_part[:, :], pattern=[[0, 1]], channel_multiplier=1,
            allow_small_or_imprecise_dtypes=True,
        )
        ident = const_pool.tile([P, P], F32)
        nc.gpsimd.tensor_scalar(
            out=ident[:, :],
            in0=iota_free[:, :],
            scalar1=iota_part[:, 0:1],
            scalar2=None,
            op0=mybir.AluOpType.is_equal,
        )
        # token-id table: score column i' = c*128 + j holds token j*IC + c
        iota_f = const_pool.tile([P, IC, P], F32)
        nc.gpsimd.iota(
            iota_f[:, :, :], pattern=[[1, IC], [IC, P]], channel_multiplier=0,
            allow_small_or_imprecise_dtypes=True,
        )

        # ---- text load + transpose (prologue) ----
        # partition p slot c -> t = p*TC + c  (contiguous 4KB per partition)
        text_nat = const_pool.tile([P, TC, D], F32)
        nc.sync.dma_start(
            out=text_nat[:, :, :],
            in_=text_in[:, :].rearrange("(p c) d -> p (c d)", c=TC),
        )
        # textT is rounded to float32r (11-bit mantissa); textT_lo holds the
        # rounding residual so text contributes ~22 exact bits to the scores
        textT = const_pool.tile([P, DC, T], F32R)
        textT_lo = const_pool.tile([P, DC, T], F32R)
        for dc in range(DC):
            ptile = psum_tr.tile([P, T], F32, tag="ptile")
            for c in range(TC):
                # block column j of the transpose is partition j, i.e.
                # true t = j*TC + c lands at textT position c*128 + j
                nc.tensor.transpose(
                    out=ptile[:, c * P : (c + 1) * P],
                    in_=text_nat[:, c, dc * P : (dc + 1) * P],
                    identity=ident[:, :],
                )
            nc.scalar.copy(out=textT[:, dc, :], in_=ptile[:, :])
            nc.vector.tensor_tensor(
                out=textT_lo[:, dc, :],
                in0=ptile[:, :],
                in1=textT[:, dc, :].bitcast(F32),
                op=mybir.AluOpType.subtract,
            )

        img_flat = img_in[:, :, :].rearrange("z i d -> (z i) d")

        # ---- main loop over image groups ----
        for zg in range(Z // ZG):
            gath = gath_pool.tile([P, TC, ZG, D], F32)
            for zz in range(ZG):
                z = zg * ZG + zz
                # token i = p*IC + c at partition p slot c (8KB runs)
                img_nat = img_pool.tile([P, IC, D], F32)
                nc.sync.dma_start(
                    out=img_nat[:, :, :],
                    in_=img_in[z, :, :].rearrange("(p c) d -> p (c d)", c=IC),
                )

                # per-token sum of squares on the scalar engine
                ssq = small_pool.tile([P, IC], F32, tag="ssq")
                sq_junk = junk_pool.tile([P, D], F32, tag="sqj")
                for c in range(IC):
                    nc.scalar.activation(
                        out=sq_junk[:, :],
                        in_=img_nat[:, c, :],
                        func=mybir.ActivationFunctionType.Square,
                        accum_out=ssq[:, c : c + 1],
                    )
                norm = small_pool.tile([P, IC], F32, tag="norm")
                nc.scalar.activation(
                    out=norm[:, :], in_=ssq[:, :],
                    func=mybir.ActivationFunctionType.Sqrt,
                )
                inv_norm = small_pool.tile([P, IC], F32, tag="inv")
                nc.vector.reciprocal(out=inv_norm[:, :], in_=norm[:, :])

                # normalize image rows (per-partition scalar per slot)
                img_s = imgs_pool.tile([P, IC, D], F32)
                for c in range(IC):
                    nc.vector.tensor_scalar(
                        out=img_s[:, c, :],
                        in0=img_nat[:, c, :],
                        scalar1=inv_norm[:, c : c + 1],
                        scalar2=None,
                        op0=mybir.AluOpType.mult,
                    )

                # transpose normalized image to [d, i'] layout
                # (column c*128+j holds token j*IC+c); imgT_lo carries the
                # float32r rounding residual for the exact-score correction
                imgT = imgT_pool.tile([P, DC, I], F32R)
                for dc in range(DC):
                    ptile = psum_tr.tile([P, I], F32, tag="ptile")
                    for c in range(IC):
                        nc.tensor.transpose(
                            out=ptile[:, c * P : (c + 1) * P],
                            in_=img_s[:, c, dc * P : (dc + 1) * P],
                            identity=ident[:, :],
                        )
                    nc.scalar.copy(out=imgT[:, dc, :], in_=ptile[:, :])

                # scores + row max + argmax per text chunk
                idxf = idx_pool.tile([P, TC], F32, tag="idxf")
                for tcx in range(TC):
                    stile = psum_sc.tile([P, I], F32)
                    for ih in range(2):
                        # exact scores: t_hi*i_hi + t_lo*i_hi + t_hi*i_lo
                        # (t_lo*i_lo ~ 2^-44 relative, negligible)
                        mms = [
                            (textT, imgT, dc) for dc in range(DC)
                        ] + [
                            (textT_lo, imgT, dc) for dc in range(DC)
                        ]
                        for k, (lh, rh, dc) in enumerate(mms):
                            nc.tensor.matmul(
                                out=stile[:, ih * 512 : (ih + 1) * 512],
                                lhsT=lh[:, dc, tcx * P : (tcx + 1) * P],
                                rhs=rh[:, dc, ih * 512 : (ih + 1) * 512],
                                start=(k == 0),
                                stop=(k == len(mms) - 1),
                            )
                    # both the row-max and the argmax read the PSUM
                    # scores directly; no SBUF eviction needed
                    m = small_pool.tile([P, 1], F32, tag="m")
                    rmax_junk = junk_pool.tile([P, I], F32, tag="rmx")
                    nc.vector.tensor_scalar(
                        out=rmax_junk[:, :],
                        in0=stile[:, :],
                        scalar1=1.0,
                        scalar2=None,
                        op0=mybir.AluOpType.mult,
                        op1=mybir.AluOpType.max,
                        accum_out=m[:, :],
                    )
                    # argmax: sum(token_id * (scores == max))
                    amx_junk = junk_pool.tile([P, I], F32, tag="amx")
                    nc.vector.scalar_tensor_tensor(
                        out=amx_junk[:, :],
                        in0=stile[:, :],
                        scalar=m[:, 0:1],
                        in1=iota_f[:, :, :].rearrange("p a b -> p (a b)"),
                        op0=mybir.AluOpType.is_equal,
                        op1=mybir.AluOpType.mult,
                        accum_out=idxf[:, tcx : tcx + 1],
                    )

                # convert to global row index (z*I + i) as uint32
                idx_u = idx_pool.tile([P, TC], U32, tag="idxu")
                nc.vector.tensor_scalar(
                    out=idx_u[:, :],
                    in0=idxf[:, :],
                    scalar1=float(z * I),
                    scalar2=None,
                    op0=mybir.AluOpType.add,
                )

                # gather raw image rows from DRAM (one offset per partition)
                for tcx in range(TC):
                    nc.gpsimd.indirect_dma_start(
                        out=gath[:, tcx, zz, :],
                        out_offset=None,
                        in_=img_flat,
                        in_offset=IndirectOffsetOnAxis(
                            ap=idx_u[:, tcx : tcx + 1], axis=0
                        ),
                        bounds_check=Z * I - 1,
                        oob_is_err=False,
                    )

            # batched store for this z-group:
            # out[t = j*TC + c, zg*ZG + zz, :] = gath[j, c, zz, :]
            nc.sync.dma_start(
                out=out_p[:, :, :].rearrange(
                    "(j c) (g zz) d -> j c g zz d", c=TC, zz=ZG
                )[:, :, zg, :, :],
                in_=gath[:, :, :, :],
            )

    nc.finalize()
    return nc


_NC_CACHE = None


def _get_nc():
    global _NC_CACHE
    if _NC_CACHE is None:
        _NC_CACHE = build_nc()
    return _NC_CACHE


def kernel(text_embeddings: np.ndarray, image_embeddings: np.ndarray) -> np.ndarray:
    text = np.ascontiguousarray(np.asarray(text_embeddings, dtype=np.float32))
    image = np.ascontiguousarray(np.asarray(image_embeddings, dtype=np.float32))
    assert text.shape == (B, T, D), text.shape
    assert image.shape == (B, Z, I, D), image.shape

    nc = _get_nc()
    in_maps = [{"text": text[b], "image": image[b]} for b in range(B)]
    res = run_bass_kernel_spmd(nc, in_maps, core_ids=list(range(N_CORES)))
    out = np.stack(
        [np.asarray(res.results[b]["out"]).reshape(T, Z, D) for b in range(B)],
        axis=0,
    )
    return out


if __name__ == "__main__":
    rng = np.random.default_rng(0)
    t = rng.standard_normal((B, T, D), dtype=np.float32)
    im = rng.standard_normal((B, Z, I, D), dtype=np.float32)
    o = kernel(text_embeddings=t, image_embeddings=im)
    print("out", o.shape, o.dtype)
